# revision 12
# baseline (speedup 1.0000x reference)
"""Causal self-attention on 8 TRN2 NeuronCores.

Sharding: core c = (batch b = c//2, head-group g = c%2).  Each core computes
the full attention for one batch and 8 of the 16 heads (column-sharded
Wq/Wk/Wv, row-sharded Wproj), producing a partial output projection; the two
partials per batch are summed on the host (the row-parallel all-reduce).

Matmul operands are bf16 (fp32 psum accumulation).  Per-core dataflow:
  xT[c_in, t]  (host pre-transposed, bf16)
  qT/kT[cq, t] = Wq/Wk^T @ xT          (pair-packed: 2 heads per 128-part tile)
  v[t, cv]     = x @ Wv                (stored [t, head, 65] with ones column)
  scoresT[k,q] = k @ qT  per head      (row-group-packed pair matmuls, K=64)
  expT         = exp(0.125*scoresT); diagonal blocks masked post-exp by a
                 0/1 lower-triangle multiply on GpSimd
  outT[dv,q],sums[q] = [v|1].T @ expT  (psum accumulate over k tiles)
  outT_scaled  = outT * (1/sums)       (broadcast via K=1 outer-product matmul
                                        + reciprocal_approx_fast)
  y_partial    = outT_scaled.T @ Wproj_rows

The attention inner loop is paced by ScalarE (exp); remaining QKV / output
projection matmul groups are generators "pumped" between attention steps so
the TensorE stream stays dense and the HAM clock stays at 2.4 GHz.
"""

import numpy as np
import ml_dtypes
from contextlib import ExitStack

import concourse.bass as bass
import concourse.tile as tile
from concourse import bacc, mybir
from concourse.bass import ts
from concourse.bass_utils import run_bass_kernel_spmd

F32 = mybir.dt.float32
BF16 = mybir.dt.bfloat16
AF = mybir.ActivationFunctionType

N_CORES = 8
T = 1024
C = 1024
D = 64          # head dim
HL = 8          # heads per core
CL = HL * D     # 512 local channels
NKT = 8         # k (key) tiles of 128
NPAIR = 4       # head pairs per core

_CACHE = {}


def _build():
    nc = bacc.Bacc("TRN2", target_bir_lowering=False, debug=False,
                   num_devices=N_CORES)
    xt = nc.dram_tensor("xt", [C, T], BF16, kind="ExternalInput").ap()
    wq = nc.dram_tensor("wq", [C, CL], BF16, kind="ExternalInput").ap()
    wk = nc.dram_tensor("wk", [C, CL], BF16, kind="ExternalInput").ap()
    wv = nc.dram_tensor("wv", [C, CL], BF16, kind="ExternalInput").ap()
    wp = nc.dram_tensor("wp", [CL, C], BF16, kind="ExternalInput").ap()
    # 0/1 keep-mask: tri[p, f] = 1 where f >= p
    tri = nc.dram_tensor("tri", [128, 128], BF16, kind="ExternalInput").ap()
    ones_a = nc.dram_tensor("ones_a", [1, 64], BF16, kind="ExternalInput").ap()
    ones_b = nc.dram_tensor("ones_b", [128, 8], BF16, kind="ExternalInput").ap()
    y = nc.dram_tensor("y", [T, C], F32, kind="ExternalOutput").ap()

    xt_r = xt.rearrange("(kt p) t -> kt p t", p=128)
    w_r = {n: w.rearrange("(kt p) n -> kt p n", p=128)
           for n, w in (("wq", wq), ("wk", wk), ("wv", wv))}

    with tile.TileContext(nc) as tc, ExitStack() as ctx:
        const = ctx.enter_context(tc.tile_pool(name="const", bufs=1))
        big = ctx.enter_context(tc.tile_pool(name="big", bufs=1))
        ps_main = ctx.enter_context(
            tc.tile_pool(name="ps_main", bufs=4, space="PSUM"))
        ps_out = ctx.enter_context(
            tc.tile_pool(name="ps_out", bufs=2, space="PSUM"))
        sb_exp = ctx.enter_context(tc.tile_pool(name="sb_exp", bufs=8))
        sb_tmp = ctx.enter_context(tc.tile_pool(name="sb_tmp", bufs=2))
        sb_y = ctx.enter_context(tc.tile_pool(name="sb_y", bufs=3))

        # ---- load phase ----
        # xt chunks on the scalar HWDGE queue, weights chunked on sync,
        # constants via gpsimd SWDGE: three parallel issue paths.
        xt_sb = []
        for kt in range(NKT):
            xc = big.tile([128, T], BF16, name=f"xt{kt}")
            nc.scalar.dma_start(out=xc[:], in_=xt_r[kt])
            xt_sb.append(xc)
        w_sb3 = {}
        for nm in ("wq", "wk", "wv"):
            wt3 = big.tile([128, NKT, CL], BF16, name=f"{nm}_sb")
            nc.sync.dma_start(out=wt3[:], in_=w_r[nm].rearrange(
                "kt p n -> p kt n"))
            w_sb3[nm] = wt3
        wq_sb = [w_sb3["wq"][:, kt, :] for kt in range(NKT)]
        wk_sb = [w_sb3["wk"][:, kt, :] for kt in range(NKT)]
        wv_sb = [w_sb3["wv"][:, kt, :] for kt in range(NKT)]
        wp_sb = big.tile([128, NPAIR, C], BF16)
        nc.sync.dma_start(
            out=wp_sb[:], in_=wp.rearrange("(r p) n -> p r n", p=128))
        tri_sb = const.tile([128, 128], BF16)
        nc.sync.dma_start(out=tri_sb[:], in_=tri)
        ones_a_sb = const.tile([65, 64], BF16)
        nc.sync.dma_start(out=ones_a_sb[64:65, :], in_=ones_a)
        ones_b_sb = const.tile([128, 8], BF16)
        nc.sync.dma_start(out=ones_b_sb[:], in_=ones_b)

        qT_sb = big.tile([128, NPAIR, T], BF16)
        kT_sb = big.tile([128, NPAIR, T], BF16)
        v_sb = big.tile([128, NKT, HL, D + 1], BF16)
        projT_sb = big.tile([128, NPAIR, T], BF16)

        # ---- PE work generators ----
        def qkv_group(dst, w_sb, m, nt):
            ps = ps_main.tile([128, 512], F32, name="ps")
            for kt in range(NKT):
                nc.tensor.matmul(
                    ps[:], w_sb[kt][:, ts(m, 128)],
                    xt_sb[kt][:, ts(nt, 512)],
                    start=(kt == 0), stop=(kt == NKT - 1))
                if kt % 2 == 1:
                    yield
            nc.vector.tensor_copy(dst[:, m, ts(nt, 512)], ps[:])

        def v_group(tt):
            ps = ps_main.tile([128, 512], F32, name="ps")
            for kt in range(NKT):
                nc.tensor.matmul(
                    ps[:], xt_sb[kt][:, ts(tt, 128)], wv_sb[kt],
                    start=(kt == 0), stop=(kt == NKT - 1))
                if kt % 2 == 1:
                    yield
            nc.vector.tensor_copy(
                v_sb[:, tt, :, 0:D],
                ps[:].rearrange("p (h d) -> p h d", h=HL))
            nc.vector.tensor_copy(v_sb[:, tt, :, D], ones_b_sb[:])

        def proj_group(q0, tt2, n2):
            ps = ps_main.tile([128, 512], F32, name="ps")
            for r in range(NPAIR):
                nc.tensor.matmul(
                    ps[:],
                    projT_sb[:, r, q0 + 128 * tt2:q0 + 128 * (tt2 + 1)],
                    wp_sb[:, r, ts(n2, 512)],
                    start=(r == 0), stop=(r == NPAIR - 1))
                if r % 2 == 1:
                    yield
            yt = sb_y.tile([128, 512], F32)
            nc.vector.tensor_copy(yt[:], ps[:])
            nc.sync.dma_start(
                out=y[q0 + 128 * tt2:q0 + 128 * (tt2 + 1), ts(n2, 512)],
                in_=yt[:])

        fillers = []  # [tag, generator]

        def pump(n):
            while n > 0 and fillers:
                tag, g = fillers[0]
                try:
                    next(g)
                    n -= 1
                except StopIteration:
                    fillers.pop(0)

        def flush(tags):
            i = 0
            while i < len(fillers):
                tag, g = fillers[i]
                if tag in tags:
                    for _ in g:
                        pass
                    fillers.pop(i)
                else:
                    i += 1

        # ---- QKV upfront: only pair-0 q(n1)/k; everything else is filler
        for _ in qkv_group(qT_sb, wq_sb, 0, 1):
            pass
        for nt in range(2):
            for _ in qkv_group(kT_sb, wk_sb, 0, nt):
                pass
        for tt in range(NKT):
            fillers.append((f"v{tt}", v_group(tt)))
        for m in range(1, NPAIR):
            fillers.append((f"k{m}n0", qkv_group(kT_sb, wk_sb, m, 0)))
            fillers.append((f"k{m}n1", qkv_group(kT_sb, wk_sb, m, 1)))
            fillers.append((f"q{m}n1", qkv_group(qT_sb, wq_sb, m, 1)))
        fillers.append(("q0n0", qkv_group(qT_sb, wq_sb, 0, 0)))
        for m in range(1, NPAIR):
            fillers.append((f"q{m}n0", qkv_group(qT_sb, wq_sb, m, 0)))

        # ---- attention ----
        def make_tail(m, outAB, q0):
            def tail():
                rrs = []
                for hh in range(2):
                    rr = sb_tmp.tile([65, 512], BF16, name="rr")
                    nc.vector.tensor_copy(rr[64:65, :], outAB[64:65, hh, :])
                    rrs.append(rr)
                pump(2)
                bcs = []
                for hh in range(2):
                    bc = ps_main.tile([64, 512], F32, name="ps")
                    nc.tensor.matmul(
                        bc[:], ones_a_sb[64:65, :], rrs[hh][64:65, :],
                        start=True, stop=True, tile_position=(64, 0))
                    bcs.append(bc)
                pump(1)
                for hh in range(2):
                    bcr = sb_tmp.tile([64, 512], F32, name="bcr")
                    nc.vector.reciprocal_approx_fast(out=bcr[:], in_=bcs[hh][:])
                    if hh == 0:
                        nc.vector.tensor_mul(
                            projT_sb[0:64, m, q0:q0 + 512],
                            outAB[0:64, 0, :], bcr[:])
                    else:
                        t2 = sb_tmp.tile([64, 512], BF16, name="t2")
                        nc.vector.tensor_mul(
                            t2[:], outAB[0:64, 1, :], bcr[:])
                        nc.sync.dma_start(
                            out=projT_sb[64:128, m, q0:q0 + 512],
                            in_=t2[:])
            return tail

        for qt in (1, 0):
            q0 = 512 * qt
            pend_tail = None
            for m in range(NPAIR):
                flush({f"q{m}n{qt}", f"k{m}n0", f"k{m}n{qt}"})
                kts = list(range(4 * qt + 4))
                outAB = ps_out.tile([65, 2, 512], F32)
                pend = None
                for i in list(range(len(kts))) + [None]:
                    if i is not None:
                        kt = kts[i]
                        flush({f"v{kt}"})
                        off = max(0, 128 * kt - q0)
                        w = 512 - off
                        qcols = slice(q0 + off, q0 + 512)
                        cur = []
                        for hh, po in ((0, 0), (1, 64)):
                            s = ps_main.tile([128, 512], F32, name="ps")[:, :w]
                            nc.tensor.matmul(
                                s,
                                kT_sb[po:po + 64, m, ts(kt, 128)],
                                qT_sb[po:po + 64, m, qcols],
                                start=True, stop=True,
                                tile_position=(po, 0))
                            e = sb_exp.tile([128, 512], BF16, name="et")[:, :w]
                            nc.scalar.activation(e, s, AF.Exp, scale=0.125)
                            if kt >= 4 * qt:  # diagonal: zero upper triangle
                                nc.gpsimd.tensor_mul(
                                    e[:, 0:128], e[:, 0:128], tri_sb[:])
                            cur.append(e)
                        cur = (cur, i, off)
                    else:
                        cur = None
                    pump(3 if qt == 0 else 2)
                    if pend is not None:
                        (eAB, pi, poff) = pend
                        for hh in range(2):
                            nc.tensor.matmul(
                                outAB[0:65, hh, poff:512],
                                v_sb[:, kts[pi], 2 * m + hh, :],
                                eAB[hh],
                                start=(pi == 0), stop=(pi == len(kts) - 1))
                    pend = cur
                    # previous pair's normalize, interleaved into this loop
                    if pend_tail is not None and (i == 1 or i is None):
                        pend_tail()
                        pend_tail = None
                pend_tail = make_tail(m, outAB, q0)
            pend_tail()
            pend_tail = None
            # queue this q-half's projection as PE filler for the next phase
            for tt2 in range(4):
                for n2 in range(2):
                    fillers.append((f"p{qt}", proj_group(q0, tt2, n2)))
        # drain remaining projection work
        while fillers:
            pump(len(fillers) * 8)

    nc.compile()
    return nc


def _program():
    if "nc" not in _CACHE:
        _CACHE["nc"] = _build()
    return _CACHE["nc"]


def _bf(a):
    return np.ascontiguousarray(a).astype(ml_dtypes.bfloat16)


def _in_maps(x, Wq, Wk, Wv, Wproj):
    tri = np.triu(np.ones((128, 128), dtype=np.float32))  # tri[p,f]=1, f>=p
    tri = np.ascontiguousarray(tri).astype(ml_dtypes.bfloat16)
    ones_a = np.ones((1, 64), dtype=ml_dtypes.bfloat16)
    ones_b = np.ones((128, 8), dtype=ml_dtypes.bfloat16)
    maps = []
    for c in range(N_CORES):
        b, g = c // 2, c % 2
        sl = slice(CL * g, CL * (g + 1))
        maps.append({
            "xt": _bf(x[b].T),
            "wq": _bf(Wq[:, sl]),
            "wk": _bf(Wk[:, sl]),
            "wv": _bf(Wv[:, sl]),
            "wp": _bf(Wproj[sl, :]),
            "tri": tri,
            "ones_a": ones_a,
            "ones_b": ones_b,
        })
    return maps


def run(x, Wq, Wk, Wv, Wproj, trace=False, **kwargs):
    nc = _program()
    maps = _in_maps(np.asarray(x, dtype=np.float32),
                    np.asarray(Wq, dtype=np.float32),
                    np.asarray(Wk, dtype=np.float32),
                    np.asarray(Wv, dtype=np.float32),
                    np.asarray(Wproj, dtype=np.float32))
    res = run_bass_kernel_spmd(nc, maps, core_ids=list(range(N_CORES)),
                               trace=trace, **kwargs)
    B = 4
    out = np.empty((B, T, C), dtype=np.float32)
    for b in range(B):
        out[b] = res.results[2 * b]["y"] + res.results[2 * b + 1]["y"]
    return out, res


def kernel(x, Wq, Wk, Wv, Wproj):
    out, _ = run(x, Wq, Wk, Wv, Wproj)
    return out


# revision 13
# speedup vs baseline: 1.0196x; 1.0196x over previous
"""Causal self-attention on 8 TRN2 NeuronCores.

Sharding: core c = (batch b = c//2, head-group g = c%2).  Each core computes
the full attention for one batch and 8 of the 16 heads (column-sharded
Wq/Wk/Wv, row-sharded Wproj), producing a partial output projection; the two
partials per batch are summed on the host (the row-parallel all-reduce).

Matmul operands are bf16 (fp32 psum accumulation).  Per-core dataflow:
  xT[c_in, t]  (host pre-transposed, bf16)
  qT/kT[cq, t] = Wq/Wk^T @ xT          (pair-packed: 2 heads per 128-part tile)
  v[t, cv]     = x @ Wv                (stored [t, head, 65] with ones column)
  scoresT[k,q] = k @ qT  per head      (row-group-packed pair matmuls, K=64)
  expT         = exp(0.125*scoresT); diagonal blocks masked post-exp by a
                 0/1 lower-triangle multiply on GpSimd
  outT[dv,q],sums[q] = [v|1].T @ expT  (psum accumulate over k tiles)
  outT_scaled  = outT * (1/sums)       (broadcast via K=1 outer-product matmul
                                        + reciprocal_approx_fast)
  y_partial    = outT_scaled.T @ Wproj_rows

The attention inner loop is paced by ScalarE (exp); remaining QKV / output
projection matmul groups are generators "pumped" between attention steps so
the TensorE stream stays dense and the HAM clock stays at 2.4 GHz.
"""

import numpy as np
import ml_dtypes
from contextlib import ExitStack

import concourse.bass as bass
import concourse.tile as tile
from concourse import bacc, mybir
from concourse.bass import ts
from concourse.bass_utils import run_bass_kernel_spmd

F32 = mybir.dt.float32
BF16 = mybir.dt.bfloat16
AF = mybir.ActivationFunctionType

N_CORES = 8
T = 1024
C = 1024
D = 64          # head dim
HL = 8          # heads per core
CL = HL * D     # 512 local channels
NKT = 8         # k (key) tiles of 128
NPAIR = 4       # head pairs per core

_CACHE = {}


def _build():
    nc = bacc.Bacc("TRN2", target_bir_lowering=False, debug=False,
                   num_devices=N_CORES)
    xt = nc.dram_tensor("xt", [C, T], BF16, kind="ExternalInput").ap()
    wq = nc.dram_tensor("wq", [C, CL], BF16, kind="ExternalInput").ap()
    wk = nc.dram_tensor("wk", [C, CL], BF16, kind="ExternalInput").ap()
    wv = nc.dram_tensor("wv", [C, CL], BF16, kind="ExternalInput").ap()
    wp = nc.dram_tensor("wp", [CL, C], BF16, kind="ExternalInput").ap()
    # 0/1 keep-mask: tri[p, f] = 1 where f >= p
    tri = nc.dram_tensor("tri", [128, 128], BF16, kind="ExternalInput").ap()
    ones_a = nc.dram_tensor("ones_a", [1, 64], BF16, kind="ExternalInput").ap()
    ones_b = nc.dram_tensor("ones_b", [128, 8], BF16, kind="ExternalInput").ap()
    y = nc.dram_tensor("y", [T, C], F32, kind="ExternalOutput").ap()

    xt_r = xt.rearrange("(kt p) t -> kt p t", p=128)
    w_r = {n: w.rearrange("(kt p) n -> kt p n", p=128)
           for n, w in (("wq", wq), ("wk", wk), ("wv", wv))}

    with tile.TileContext(nc) as tc, ExitStack() as ctx:
        const = ctx.enter_context(tc.tile_pool(name="const", bufs=1))
        big = ctx.enter_context(tc.tile_pool(name="big", bufs=1))
        ps_main = ctx.enter_context(
            tc.tile_pool(name="ps_main", bufs=4, space="PSUM"))
        ps_out = ctx.enter_context(
            tc.tile_pool(name="ps_out", bufs=2, space="PSUM"))
        sb_exp = ctx.enter_context(tc.tile_pool(name="sb_exp", bufs=8))
        sb_tmp = ctx.enter_context(tc.tile_pool(name="sb_tmp", bufs=2))
        sb_y = ctx.enter_context(tc.tile_pool(name="sb_y", bufs=3))

        # ---- load phase ----
        # xt chunks on the scalar HWDGE queue, weights chunked on sync,
        # constants via gpsimd SWDGE: three parallel issue paths.
        xt_sb = []
        for kt in range(NKT):
            xc = big.tile([128, T], BF16, name=f"xt{kt}")
            nc.scalar.dma_start(out=xc[:], in_=xt_r[kt])
            xt_sb.append(xc)
        w_sb3 = {}
        for nm in ("wq", "wk", "wv"):
            wt3 = big.tile([128, NKT, CL], BF16, name=f"{nm}_sb")
            nc.sync.dma_start(out=wt3[:], in_=w_r[nm].rearrange(
                "kt p n -> p kt n"))
            w_sb3[nm] = wt3
        wq_sb = [w_sb3["wq"][:, kt, :] for kt in range(NKT)]
        wk_sb = [w_sb3["wk"][:, kt, :] for kt in range(NKT)]
        wv_sb = [w_sb3["wv"][:, kt, :] for kt in range(NKT)]
        wp_sb = big.tile([128, NPAIR, C], BF16)
        nc.sync.dma_start(
            out=wp_sb[:], in_=wp.rearrange("(r p) n -> p r n", p=128))
        tri_sb = const.tile([128, 128], BF16)
        nc.sync.dma_start(out=tri_sb[:], in_=tri)
        ones_a_sb = const.tile([65, 64], BF16)
        nc.sync.dma_start(out=ones_a_sb[64:65, :], in_=ones_a)
        ones_b_sb = const.tile([128, 8], BF16)
        nc.sync.dma_start(out=ones_b_sb[:], in_=ones_b)

        qT_sb = big.tile([128, NPAIR, T], BF16)
        kT_sb = big.tile([128, NPAIR, T], BF16)
        v_sb = big.tile([128, NKT, HL, D + 1], BF16)
        projT_sb = big.tile([128, NPAIR, T], BF16)

        # ---- PE work generators ----
        def qkv_group(dst, w_sb, m, nt):
            ps = ps_main.tile([128, 512], F32, name="ps")
            for kt in range(NKT):
                nc.tensor.matmul(
                    ps[:], w_sb[kt][:, ts(m, 128)],
                    xt_sb[kt][:, ts(nt, 512)],
                    start=(kt == 0), stop=(kt == NKT - 1))
                if kt % 2 == 1:
                    yield
            nc.vector.tensor_copy(dst[:, m, ts(nt, 512)], ps[:])

        def v_group(tt):
            ps = ps_main.tile([128, 512], F32, name="ps")
            for kt in range(NKT):
                nc.tensor.matmul(
                    ps[:], xt_sb[kt][:, ts(tt, 128)], wv_sb[kt],
                    start=(kt == 0), stop=(kt == NKT - 1))
                if kt % 2 == 1:
                    yield
            nc.vector.tensor_copy(
                v_sb[:, tt, :, 0:D],
                ps[:].rearrange("p (h d) -> p h d", h=HL))
            nc.vector.tensor_copy(v_sb[:, tt, :, D], ones_b_sb[:])

        def proj_group(q0, tt2, n2):
            ps = ps_main.tile([128, 512], F32, name="ps")
            for r in range(NPAIR):
                nc.tensor.matmul(
                    ps[:],
                    projT_sb[:, r, q0 + 128 * tt2:q0 + 128 * (tt2 + 1)],
                    wp_sb[:, r, ts(n2, 512)],
                    start=(r == 0), stop=(r == NPAIR - 1))
                if r % 2 == 1:
                    yield
            yt = sb_y.tile([128, 512], F32)
            nc.vector.tensor_copy(yt[:], ps[:])
            nc.sync.dma_start(
                out=y[q0 + 128 * tt2:q0 + 128 * (tt2 + 1), ts(n2, 512)],
                in_=yt[:])

        fillers = []  # [tag, generator]

        def pump(n):
            while n > 0 and fillers:
                tag, g = fillers[0]
                try:
                    next(g)
                    n -= 1
                except StopIteration:
                    fillers.pop(0)

        def flush(tags):
            i = 0
            while i < len(fillers):
                tag, g = fillers[i]
                if tag in tags:
                    for _ in g:
                        pass
                    fillers.pop(i)
                else:
                    i += 1

        # ---- QKV upfront: only pair-0 q(n1)/k; everything else is filler
        for _ in qkv_group(qT_sb, wq_sb, 0, 1):
            pass
        for nt in range(2):
            for _ in qkv_group(kT_sb, wk_sb, 0, nt):
                pass
        for tt in range(NKT):
            fillers.append((f"v{tt}", v_group(tt)))
        for m in range(1, NPAIR):
            fillers.append((f"k{m}n0", qkv_group(kT_sb, wk_sb, m, 0)))
            fillers.append((f"k{m}n1", qkv_group(kT_sb, wk_sb, m, 1)))
            fillers.append((f"q{m}n1", qkv_group(qT_sb, wq_sb, m, 1)))
        fillers.append(("q0n0", qkv_group(qT_sb, wq_sb, 0, 0)))
        for m in range(1, NPAIR):
            fillers.append((f"q{m}n0", qkv_group(qT_sb, wq_sb, m, 0)))

        # ---- attention ----
        def make_tail(m, outAB, q0):
            def tail():
                rrs = []
                for hh in range(2):
                    rr = sb_tmp.tile([65, 512], BF16, name="rr")
                    nc.vector.tensor_copy(rr[64:65, :], outAB[64:65, hh, :])
                    rrs.append(rr)
                pump(2)
                bcs = []
                for hh in range(2):
                    bc = ps_main.tile([64, 512], F32, name="ps")
                    nc.tensor.matmul(
                        bc[:], ones_a_sb[64:65, :], rrs[hh][64:65, :],
                        start=True, stop=True, tile_position=(64, 0))
                    bcs.append(bc)
                pump(1)
                for hh in range(2):
                    bcr = sb_tmp.tile([64, 512], F32, name="bcr")
                    nc.vector.reciprocal_approx_fast(out=bcr[:], in_=bcs[hh][:])
                    if hh == 0:
                        nc.vector.tensor_mul(
                            projT_sb[0:64, m, q0:q0 + 512],
                            outAB[0:64, 0, :], bcr[:])
                    else:
                        t2 = sb_tmp.tile([64, 512], BF16, name="t2")
                        nc.vector.tensor_mul(
                            t2[:], outAB[0:64, 1, :], bcr[:])
                        nc.sync.dma_start(
                            out=projT_sb[64:128, m, q0:q0 + 512],
                            in_=t2[:])
            return tail

        for qt in (1, 0):
            q0 = 512 * qt
            pend_tail = None
            for m in range(NPAIR):
                flush({f"q{m}n{qt}", f"k{m}n0", f"k{m}n{qt}"})
                kts = list(range(4 * qt + 4))
                outAB = ps_out.tile([65, 2, 512], F32)
                pend = None
                for i in list(range(len(kts))) + [None]:
                    if i is not None:
                        kt = kts[i]
                        flush({f"v{kt}", f"v{min(kt + 2, NKT - 1)}"})
                        off = max(0, 128 * kt - q0)
                        w = 512 - off
                        qcols = slice(q0 + off, q0 + 512)
                        cur = []
                        for hh, po in ((0, 0), (1, 64)):
                            s = ps_main.tile([128, 512], F32, name="ps")[:, :w]
                            nc.tensor.matmul(
                                s,
                                kT_sb[po:po + 64, m, ts(kt, 128)],
                                qT_sb[po:po + 64, m, qcols],
                                start=True, stop=True,
                                tile_position=(po, 0))
                            e = sb_exp.tile([128, 512], BF16, name="et")[:, :w]
                            nc.scalar.activation(e, s, AF.Exp, scale=0.125)
                            if kt >= 4 * qt:  # diagonal: zero upper triangle
                                nc.gpsimd.tensor_mul(
                                    e[:, 0:128], e[:, 0:128], tri_sb[:])
                            cur.append(e)
                        cur = (cur, i, off)
                    else:
                        cur = None
                    pump(3 if qt == 0 else 2)
                    if pend is not None:
                        (eAB, pi, poff) = pend
                        for hh in range(2):
                            nc.tensor.matmul(
                                outAB[0:65, hh, poff:512],
                                v_sb[:, kts[pi], 2 * m + hh, :],
                                eAB[hh],
                                start=(pi == 0), stop=(pi == len(kts) - 1))
                    pend = cur
                    # previous pair's normalize, interleaved into this loop
                    if pend_tail is not None and (i == 1 or i is None):
                        pend_tail()
                        pend_tail = None
                pend_tail = make_tail(m, outAB, q0)
            pend_tail()
            pend_tail = None
            # queue this q-half's projection as PE filler for the next phase
            for tt2 in range(4):
                for n2 in range(2):
                    fillers.append((f"p{qt}", proj_group(q0, tt2, n2)))
        # drain remaining projection work
        while fillers:
            pump(len(fillers) * 8)

    nc.compile()
    return nc


def _program():
    if "nc" not in _CACHE:
        _CACHE["nc"] = _build()
    return _CACHE["nc"]


def _bf(a):
    return np.ascontiguousarray(a).astype(ml_dtypes.bfloat16)


def _in_maps(x, Wq, Wk, Wv, Wproj):
    tri = np.triu(np.ones((128, 128), dtype=np.float32))  # tri[p,f]=1, f>=p
    tri = np.ascontiguousarray(tri).astype(ml_dtypes.bfloat16)
    ones_a = np.ones((1, 64), dtype=ml_dtypes.bfloat16)
    ones_b = np.ones((128, 8), dtype=ml_dtypes.bfloat16)
    maps = []
    for c in range(N_CORES):
        b, g = c // 2, c % 2
        sl = slice(CL * g, CL * (g + 1))
        maps.append({
            "xt": _bf(x[b].T),
            "wq": _bf(Wq[:, sl]),
            "wk": _bf(Wk[:, sl]),
            "wv": _bf(Wv[:, sl]),
            "wp": _bf(Wproj[sl, :]),
            "tri": tri,
            "ones_a": ones_a,
            "ones_b": ones_b,
        })
    return maps


def run(x, Wq, Wk, Wv, Wproj, trace=False, **kwargs):
    nc = _program()
    maps = _in_maps(np.asarray(x, dtype=np.float32),
                    np.asarray(Wq, dtype=np.float32),
                    np.asarray(Wk, dtype=np.float32),
                    np.asarray(Wv, dtype=np.float32),
                    np.asarray(Wproj, dtype=np.float32))
    res = run_bass_kernel_spmd(nc, maps, core_ids=list(range(N_CORES)),
                               trace=trace, **kwargs)
    B = 4
    out = np.empty((B, T, C), dtype=np.float32)
    for b in range(B):
        out[b] = res.results[2 * b]["y"] + res.results[2 * b + 1]["y"]
    return out, res


def kernel(x, Wq, Wk, Wv, Wproj):
    out, _ = run(x, Wq, Wk, Wv, Wproj)
    return out


# revision 14
# speedup vs baseline: 1.0362x; 1.0163x over previous
"""Causal self-attention on 8 TRN2 NeuronCores.

Sharding: core c = (batch b = c//2, head-group g = c%2).  Each core computes
the full attention for one batch and 8 of the 16 heads (column-sharded
Wq/Wk/Wv, row-sharded Wproj), producing a partial output projection; the two
partials per batch are summed on the host (the row-parallel all-reduce).

Matmul operands are bf16 (fp32 psum accumulation).  Per-core dataflow:
  xT[c_in, t]  (host pre-transposed, bf16)
  qT/kT[cq, t] = Wq/Wk^T @ xT          (pair-packed: 2 heads per 128-part tile)
  v[t, cv]     = x @ Wv                (stored [t, head, 65] with ones column)
  scoresT[k,q] = k @ qT  per head      (row-group-packed pair matmuls, K=64)
  expT         = exp(0.125*scoresT); diagonal blocks masked post-exp by a
                 0/1 lower-triangle multiply on GpSimd
  outT[dv,q],sums[q] = [v|1].T @ expT  (psum accumulate over k tiles)
  outT_scaled  = outT * (1/sums)       (broadcast via K=1 outer-product matmul
                                        + reciprocal_approx_fast)
  y_partial    = outT_scaled.T @ Wproj_rows

The attention inner loop is paced by ScalarE (exp); remaining QKV / output
projection matmul groups are generators "pumped" between attention steps so
the TensorE stream stays dense and the HAM clock stays at 2.4 GHz.
"""

import numpy as np
import ml_dtypes
from contextlib import ExitStack

import concourse.bass as bass
import concourse.tile as tile
from concourse import bacc, mybir
from concourse.bass import ts
from concourse.bass_utils import run_bass_kernel_spmd

F32 = mybir.dt.float32
BF16 = mybir.dt.bfloat16
AF = mybir.ActivationFunctionType

N_CORES = 8
T = 1024
C = 1024
D = 64          # head dim
HL = 8          # heads per core
CL = HL * D     # 512 local channels
NKT = 8         # k (key) tiles of 128
NPAIR = 4       # head pairs per core

_CACHE = {}


def _build():
    nc = bacc.Bacc("TRN2", target_bir_lowering=False, debug=False,
                   num_devices=N_CORES)
    xt = nc.dram_tensor("xt", [C, T], BF16, kind="ExternalInput").ap()
    wq = nc.dram_tensor("wq", [C, CL], BF16, kind="ExternalInput").ap()
    wk = nc.dram_tensor("wk", [C, CL], BF16, kind="ExternalInput").ap()
    wv = nc.dram_tensor("wv", [C, CL], BF16, kind="ExternalInput").ap()
    wp = nc.dram_tensor("wp", [CL, C], BF16, kind="ExternalInput").ap()
    # 0/1 keep-mask: tri[p, f] = 1 where f >= p
    tri = nc.dram_tensor("tri", [128, 128], BF16, kind="ExternalInput").ap()
    ones_a = nc.dram_tensor("ones_a", [1, 64], BF16, kind="ExternalInput").ap()
    ones_b = nc.dram_tensor("ones_b", [128, 8], BF16, kind="ExternalInput").ap()
    y = nc.dram_tensor("y", [T, C], F32, kind="ExternalOutput").ap()

    xt_r = xt.rearrange("(kt p) t -> kt p t", p=128)
    w_r = {n: w.rearrange("(kt p) n -> kt p n", p=128)
           for n, w in (("wq", wq), ("wk", wk), ("wv", wv))}

    with tile.TileContext(nc) as tc, ExitStack() as ctx:
        const = ctx.enter_context(tc.tile_pool(name="const", bufs=1))
        big = ctx.enter_context(tc.tile_pool(name="big", bufs=1))
        ps_main = ctx.enter_context(
            tc.tile_pool(name="ps_main", bufs=4, space="PSUM"))
        ps_out = ctx.enter_context(
            tc.tile_pool(name="ps_out", bufs=2, space="PSUM"))
        sb_exp = ctx.enter_context(tc.tile_pool(name="sb_exp", bufs=8))
        sb_tmp = ctx.enter_context(tc.tile_pool(name="sb_tmp", bufs=2))
        sb_y = ctx.enter_context(tc.tile_pool(name="sb_y", bufs=3))

        # ---- load phase ----
        # xt chunks on the scalar HWDGE queue, weights chunked on sync,
        # constants via gpsimd SWDGE: three parallel issue paths.
        xt_sb, wq_sb, wk_sb, wv_sb = [], [], [], []
        for kt in range(NKT):
            wc = big.tile([128, CL], BF16, name=f"wv{kt}")
            nc.sync.dma_start(out=wc[:], in_=w_r["wv"][kt])
            wv_sb.append(wc)
            xc = big.tile([128, T], BF16, name=f"xt{kt}")
            nc.scalar.dma_start(out=xc[:], in_=xt_r[kt])
            xt_sb.append(xc)
        for kt in range(NKT):
            wc = big.tile([128, CL], BF16, name=f"wq{kt}")
            nc.sync.dma_start(out=wc[:], in_=w_r["wq"][kt])
            wq_sb.append(wc)
        for kt in range(NKT):
            wc = big.tile([128, CL], BF16, name=f"wk{kt}")
            nc.sync.dma_start(out=wc[:], in_=w_r["wk"][kt])
            wk_sb.append(wc)
        wp_sb = big.tile([128, NPAIR, C], BF16)
        nc.sync.dma_start(
            out=wp_sb[:], in_=wp.rearrange("(r p) n -> p r n", p=128))
        tri_sb = const.tile([128, 128], BF16)
        nc.sync.dma_start(out=tri_sb[:], in_=tri)
        ones_a_sb = const.tile([65, 64], BF16)
        nc.sync.dma_start(out=ones_a_sb[64:65, :], in_=ones_a)
        ones_b_sb = const.tile([128, 8], BF16)
        nc.sync.dma_start(out=ones_b_sb[:], in_=ones_b)

        qT_sb = big.tile([128, NPAIR, T], BF16)
        kT_sb = big.tile([128, NPAIR, T], BF16)
        v_sb = big.tile([128, NKT, HL, D + 1], BF16)
        projT_sb = big.tile([128, NPAIR, T], BF16)

        # ---- PE work generators ----
        def qkv_group(dst, w_sb, m, nt):
            ps = ps_main.tile([128, 512], F32, name="ps")
            for kt in range(NKT):
                nc.tensor.matmul(
                    ps[:], w_sb[kt][:, ts(m, 128)],
                    xt_sb[kt][:, ts(nt, 512)],
                    start=(kt == 0), stop=(kt == NKT - 1))
                if kt % 2 == 1:
                    yield
            nc.vector.tensor_copy(dst[:, m, ts(nt, 512)], ps[:])

        def v_group(tt):
            ps = ps_main.tile([128, 512], F32, name="ps")
            for kt in range(NKT):
                nc.tensor.matmul(
                    ps[:], xt_sb[kt][:, ts(tt, 128)], wv_sb[kt][:],
                    start=(kt == 0), stop=(kt == NKT - 1))
                if kt % 2 == 1:
                    yield
            nc.vector.tensor_copy(
                v_sb[:, tt, :, 0:D],
                ps[:].rearrange("p (h d) -> p h d", h=HL))
            nc.vector.tensor_copy(v_sb[:, tt, :, D], ones_b_sb[:])

        def proj_group(q0, tt2, n2):
            ps = ps_main.tile([128, 512], F32, name="ps")
            for r in range(NPAIR):
                nc.tensor.matmul(
                    ps[:],
                    projT_sb[:, r, q0 + 128 * tt2:q0 + 128 * (tt2 + 1)],
                    wp_sb[:, r, ts(n2, 512)],
                    start=(r == 0), stop=(r == NPAIR - 1))
                if r % 2 == 1:
                    yield
            yt = sb_y.tile([128, 512], F32)
            nc.vector.tensor_copy(yt[:], ps[:])
            nc.sync.dma_start(
                out=y[q0 + 128 * tt2:q0 + 128 * (tt2 + 1), ts(n2, 512)],
                in_=yt[:])

        fillers = []  # [tag, generator]

        def pump(n):
            while n > 0 and fillers:
                tag, g = fillers[0]
                try:
                    next(g)
                    n -= 1
                except StopIteration:
                    fillers.pop(0)

        def flush(tags):
            i = 0
            while i < len(fillers):
                tag, g = fillers[i]
                if tag in tags:
                    for _ in g:
                        pass
                    fillers.pop(i)
                else:
                    i += 1

        # ---- QKV upfront: v (all), q/k for pair 0; rest queued as filler
        for tt in range(NKT):
            for _ in v_group(tt):
                pass
        for nt in range(2):
            for _ in qkv_group(qT_sb, wq_sb, 0, nt):
                pass
        for nt in range(2):
            for _ in qkv_group(kT_sb, wk_sb, 0, nt):
                pass
        for m in range(1, NPAIR):
            fillers.append((f"k{m}n0", qkv_group(kT_sb, wk_sb, m, 0)))
            fillers.append((f"k{m}n1", qkv_group(kT_sb, wk_sb, m, 1)))
            fillers.append((f"q{m}n1", qkv_group(qT_sb, wq_sb, m, 1)))
        for m in range(1, NPAIR):
            fillers.append((f"q{m}n0", qkv_group(qT_sb, wq_sb, m, 0)))

        # ---- attention ----
        def make_tail(m, outAB, q0):
            def tail():
                rrs = []
                for hh in range(2):
                    rr = sb_tmp.tile([65, 512], BF16, name="rr")
                    nc.vector.tensor_copy(rr[64:65, :], outAB[64:65, hh, :])
                    rrs.append(rr)
                pump(2)
                bcs = []
                for hh in range(2):
                    bc = ps_main.tile([64, 512], F32, name="ps")
                    nc.tensor.matmul(
                        bc[:], ones_a_sb[64:65, :], rrs[hh][64:65, :],
                        start=True, stop=True, tile_position=(64, 0))
                    bcs.append(bc)
                pump(1)
                for hh in range(2):
                    bcr = sb_tmp.tile([64, 512], F32, name="bcr")
                    nc.vector.reciprocal_approx_fast(out=bcr[:], in_=bcs[hh][:])
                    if hh == 0:
                        nc.vector.tensor_mul(
                            projT_sb[0:64, m, q0:q0 + 512],
                            outAB[0:64, 0, :], bcr[:])
                    else:
                        t2 = sb_tmp.tile([64, 512], BF16, name="t2")
                        nc.vector.tensor_mul(
                            t2[:], outAB[0:64, 1, :], bcr[:])
                        nc.sync.dma_start(
                            out=projT_sb[64:128, m, q0:q0 + 512],
                            in_=t2[:])
            return tail

        for qt in (1, 0):
            q0 = 512 * qt
            pend_tail = None
            for m in range(NPAIR):
                flush({f"q{m}n{qt}", f"k{m}n0", f"k{m}n{qt}"})
                kts = list(range(4 * qt + 4))
                outAB = ps_out.tile([65, 2, 512], F32)
                pend = None
                for i in list(range(len(kts))) + [None]:
                    if i is not None:
                        kt = kts[i]
                        off = max(0, 128 * kt - q0)
                        w = 512 - off
                        qcols = slice(q0 + off, q0 + 512)
                        cur = []
                        for hh, po in ((0, 0), (1, 64)):
                            s = ps_main.tile([128, 512], F32, name="ps")[:, :w]
                            nc.tensor.matmul(
                                s,
                                kT_sb[po:po + 64, m, ts(kt, 128)],
                                qT_sb[po:po + 64, m, qcols],
                                start=True, stop=True,
                                tile_position=(po, 0))
                            e = sb_exp.tile([128, 512], BF16, name="et")[:, :w]
                            nc.scalar.activation(e, s, AF.Exp, scale=0.125)
                            if kt >= 4 * qt:  # diagonal: zero upper triangle
                                nc.gpsimd.tensor_mul(
                                    e[:, 0:128], e[:, 0:128], tri_sb[:])
                            cur.append(e)
                        cur = (cur, i, off)
                    else:
                        cur = None
                    pump(3 if qt == 0 else 2)
                    if pend is not None:
                        (eAB, pi, poff) = pend
                        for hh in range(2):
                            nc.tensor.matmul(
                                outAB[0:65, hh, poff:512],
                                v_sb[:, kts[pi], 2 * m + hh, :],
                                eAB[hh],
                                start=(pi == 0), stop=(pi == len(kts) - 1))
                    pend = cur
                    # previous pair's normalize, interleaved into this loop
                    if pend_tail is not None and (i == 1 or i is None):
                        pend_tail()
                        pend_tail = None
                pend_tail = make_tail(m, outAB, q0)
            pend_tail()
            pend_tail = None
            # queue this q-half's projection as PE filler for the next phase
            for tt2 in range(4):
                for n2 in range(2):
                    fillers.append((f"p{qt}", proj_group(q0, tt2, n2)))
        # drain remaining projection work
        while fillers:
            pump(len(fillers) * 8)

    nc.compile()
    return nc


def _program():
    if "nc" not in _CACHE:
        _CACHE["nc"] = _build()
    return _CACHE["nc"]


def _bf(a):
    return np.ascontiguousarray(a).astype(ml_dtypes.bfloat16)


def _in_maps(x, Wq, Wk, Wv, Wproj):
    tri = np.triu(np.ones((128, 128), dtype=np.float32))  # tri[p,f]=1, f>=p
    tri = np.ascontiguousarray(tri).astype(ml_dtypes.bfloat16)
    ones_a = np.ones((1, 64), dtype=ml_dtypes.bfloat16)
    ones_b = np.ones((128, 8), dtype=ml_dtypes.bfloat16)
    maps = []
    for c in range(N_CORES):
        b, g = c // 2, c % 2
        sl = slice(CL * g, CL * (g + 1))
        maps.append({
            "xt": _bf(x[b].T),
            "wq": _bf(Wq[:, sl]),
            "wk": _bf(Wk[:, sl]),
            "wv": _bf(Wv[:, sl]),
            "wp": _bf(Wproj[sl, :]),
            "tri": tri,
            "ones_a": ones_a,
            "ones_b": ones_b,
        })
    return maps


def run(x, Wq, Wk, Wv, Wproj, trace=False, **kwargs):
    nc = _program()
    maps = _in_maps(np.asarray(x, dtype=np.float32),
                    np.asarray(Wq, dtype=np.float32),
                    np.asarray(Wk, dtype=np.float32),
                    np.asarray(Wv, dtype=np.float32),
                    np.asarray(Wproj, dtype=np.float32))
    res = run_bass_kernel_spmd(nc, maps, core_ids=list(range(N_CORES)),
                               trace=trace, **kwargs)
    B = 4
    out = np.empty((B, T, C), dtype=np.float32)
    for b in range(B):
        out[b] = res.results[2 * b]["y"] + res.results[2 * b + 1]["y"]
    return out, res


def kernel(x, Wq, Wk, Wv, Wproj):
    out, _ = run(x, Wq, Wk, Wv, Wproj)
    return out
